# revision 22
# baseline (speedup 1.0000x reference)
"""Trainium2 Bass kernel for nn_AttnLayer (additive-attention pooling layer).

Reference computation (per batch b):
    e = e_hiddens @ We_w.T + We_b            # [S, F]
    d = Wd_w @ d_hiddens[b]                  # [F]
    h = tanh(d + e)                          # [S, F]
    s = h @ v_w[0] + v_b                     # [S]
    a = softmax(s)                           # [S]
    out[b] = a @ e_hiddens[b]                # [D]

Strategy (8 cores, data-parallel over batch B=32 -> 4 per core):
  x is pre-transposed ON HOST to [d-partition, s-free] bf16 tiles, so the
  device needs no PE transposes and no PSUM->SBUF copy stream, and DMA
  traffic halves (32 MiB/core).  Per 1024-long s-chunk:
    PE : e^T[f,s] = sum_k wet[dk,f]^T @ xt[dk,s]   (8 bf16 matmuls x 2 halves)
         scores replicated to all 128 partitions via a v-replicated
         stationary: sc[p,s] = sum_f vrep[f,p] h[f,s]  (2 matmuls)
    ACT: h = tanh(e^T + dvec_b)  ;  a = exp(sc + v_b) -> bf16 (accum_out
         gives the chunk's Z partial for free)
    WS : out_k += sum_s xt[dk,s] * a[s]  -- fused multiply+reduce
         (scalar_tensor_tensor accum_out) split DVE(k=0..2) / Pool(k=3..5),
         and tensor_tensor mult + ACT Copy-accum reduce for k=6..7,
         balancing all three engines under the DMA window.
  Softmax normalization (divide by Z) happens on host; the device returns
  unnormalized weighted sums plus per-batch Z.
"""

import numpy as np
import ml_dtypes

import concourse.bass as bass
import concourse.bacc as bacc
import concourse.mybir as mybir
import concourse.tile as tile
from concourse.bass_utils import run_bass_kernel_spmd
from concourse.dve_ops import TENSOR_TENSOR_REDUCE as TTR_OP

F32 = mybir.dt.float32
F32R = mybir.dt.float32r
BF16 = mybir.dt.bfloat16
F8 = mybir.dt.float8e4
AF = mybir.ActivationFunctionType
ALU = mybir.AluOpType
AX = mybir.AxisListType

N_CORES = 8
B, S, D, F = 32, 4096, 1024, 128
BP = B // N_CORES          # batches per core
KD = D // 128              # d-slices (partition groups)
SC = 1024                  # s-chunk (record granularity)
NCH = S // SC              # chunks per batch
NREC = BP * NCH            # records per core

# weighted-sum k-slice assignment, balanced to measured HW rates:
# DVE fused mult+reduce (TENSOR_TENSOR_REDUCE custom op) ~1.13-1.77us/slice,
# DVE plain mult ~0.6-1.2, Pool mult ~2.7-2.9, ACT Copy+accum reduce ~1.2,
# DVE tensor_reduce ~1.1 (writes no discard stream).
K_TTR = (0, 1, 2)           # DVE custom TENSOR_TENSOR_REDUCE (fused)
K_DVE_TT = (3, 4, 5)        # DVE tensor_tensor mult -> ACT Copy accum reduce
K_POOL_TT = (6, 7)          # Pool tensor_tensor mult -> ACT Copy accum reduce


def build_nc(bp=BP, s=S, d=D, f=F):
    nc = bacc.Bacc("TRN2", target_bir_lowering=False, debug=False)

    xt_dram = nc.dram_tensor("xt", [bp, NCH, 128, KD * SC], BF16,
                             kind="ExternalInput").ap()
    wet_dram = nc.dram_tensor("wet", [128, KD * f], BF16, kind="ExternalInput").ap()
    vrep_dram = nc.dram_tensor("vrep", [f, 128], BF16, kind="ExternalInput").ap()
    vbb_dram = nc.dram_tensor("vbb", [128, 1], F32, kind="ExternalInput").ap()
    web_dram = nc.dram_tensor("web", [f, 1], F32, kind="ExternalInput").ap()
    wdt_dram = nc.dram_tensor("wdt", [128, KD * f], F32R, kind="ExternalInput").ap()
    dht_dram = nc.dram_tensor("dht", [128, KD * bp], F32R, kind="ExternalInput").ap()
    out_dram = nc.dram_tensor("out", [bp, d], F32, kind="ExternalOutput").ap()
    z_dram = nc.dram_tensor("z", [1, bp], F32, kind="ExternalOutput").ap()

    with tile.TileContext(nc) as tc:
        with (
            tc.tile_pool(name="const", bufs=1) as const,
            tc.tile_pool(name="xpool", bufs=5) as xpool,
            tc.tile_pool(name="hpool", bufs=2) as hpool,
            tc.tile_pool(name="apool", bufs=3) as apool,
            tc.tile_pool(name="p2pool", bufs=3) as p2pool,
            tc.tile_pool(name="dpool", bufs=2) as dpool,
            tc.tile_pool(name="wpool", bufs=2) as wpool,
            tc.tile_pool(name="ps_e", bufs=2, space="PSUM") as ps_e,
            tc.tile_pool(name="ps_sc", bufs=1, space="PSUM") as ps_sc,
            tc.tile_pool(name="ps_m", bufs=1, space="PSUM") as ps_m,
        ):
            records = [(b, c) for b in range(bp) for c in range(NCH)]

            def load_x(i):
                b, c = records[i]
                xt = xpool.tile([128, KD, SC], BF16, tag="x", name=f"x_{b}_{c}")
                nc.sync.dma_start(
                    xt, xt_dram[b, c].rearrange("p (k s) -> p k s", k=KD))
                return xt

            xts = {i: load_x(i) for i in range(min(3, NREC))}

            # ---- constants (ordered by first use) ----
            wet_sb = const.tile([128, KD, f], BF16)
            nc.sync.dma_start(wet_sb, wet_dram.rearrange("p (k f) -> p k f", k=KD))
            vrep_sb = const.tile([f, 128], BF16)
            nc.sync.dma_start(vrep_sb, vrep_dram)
            vbb_sb = const.tile([128, 1], F32)
            nc.sync.dma_start(vbb_sb, vbb_dram)
            web_sb = const.tile([f, 1], F32)
            nc.sync.dma_start(web_sb, web_dram)
            wdt_sb = const.tile([128, KD, f], F32R)
            nc.sync.dma_start(wdt_sb, wdt_dram.rearrange("p (k f) -> p k f", k=KD))
            dht_sb = const.tile([128, KD, bp], F32R)
            nc.sync.dma_start(dht_sb, dht_dram.rearrange("p (k b) -> p k b", k=KD))
            dvec_sb = const.tile([f, bp], F32)
            zcols_sb = const.tile([128, bp * NCH], F32)
            zvals_sb = const.tile([1, bp], F32)

            state = {}

            def emm(i):
                b, c = records[i]
                e_ps = ps_e.tile([f, SC], F32, tag="e", name=f"e_{b}_{c}")
                xt = xts.pop(i)
                for k in range(KD):
                    for h2 in range(2):
                        sl = slice(h2 * 512, (h2 + 1) * 512)
                        nc.tensor.matmul(
                            e_ps[:, sl], wet_sb[:, k, :], xt[:, k, sl],
                            start=(k == 0), stop=(k == KD - 1),
                        )
                state[i] = {"e_ps": e_ps, "xt": xt}

            def tanh_s(i):
                b, c = records[i]
                e_ps = state[i].pop("e_ps")
                h_sb = hpool.tile([f, SC], BF16, tag="h", name=f"h_{b}_{c}")
                nc.scalar.activation(h_sb, e_ps, AF.Tanh, bias=dvec_sb[:, b:b + 1])
                state[i]["h"] = h_sb

            def scores(i):
                b, c = records[i]
                h_sb = state[i].pop("h")
                sc_ps = ps_sc.tile([128, SC], F32, tag="sc", name=f"sc_{b}_{c}")
                for h2 in range(2):
                    sl = slice(h2 * 512, (h2 + 1) * 512)
                    nc.tensor.matmul(sc_ps[:, sl], vrep_sb, h_sb[:, sl],
                                     start=True, stop=True)
                state[i]["sc_ps"] = sc_ps

            def expa(i):
                b, c = records[i]
                sc_ps = state[i].pop("sc_ps")
                a_bc = apool.tile([128, SC], BF16, tag="a", name=f"a_{b}_{c}")
                nc.scalar.activation(a_bc, sc_ps, AF.Exp, bias=vbb_sb,
                                     accum_out=zcols_sb[:, b * NCH + c:b * NCH + c + 1])
                state[i]["a"] = a_bc

            def get_partials(b, eng, n):
                key = (b, eng)
                if key not in partials:
                    partials[key] = wpool.tile(
                        [128, n, NCH], F32, tag=f"pt_{eng}", name=f"pt_{eng}_{b}")
                return partials[key]

            partials = {}

            def ws_produce(i):
                b, c = records[i]
                st = state.pop(i)
                xt, a_bc = st["xt"], st["a"]
                ptd = get_partials(b, "d", len(K_TTR))
                # Pool first (slowest producer), then DVE mults (feed ACT
                # early), then the fused DVE reduces.
                p2p = p2pool.tile([128, len(K_POOL_TT), SC], BF16, tag="p2p",
                                  name=f"p2p_{b}_{c}")
                for j, k in enumerate(K_POOL_TT):
                    nc.gpsimd.tensor_tensor(p2p[:, j, :], xt[:, k, :], a_bc,
                                            op=ALU.mult)
                p2d = p2pool.tile([128, len(K_DVE_TT), SC], BF16, tag="p2d",
                                  name=f"p2d_{b}_{c}")
                for j, k in enumerate(K_DVE_TT):
                    nc.vector.tensor_tensor(p2d[:, j, :], xt[:, k, :], a_bc,
                                            op=ALU.mult)
                dd = dpool.tile([128, SC], F8, tag="dd", name=f"dd_{b}_{c}")
                for j, k in enumerate(K_TTR):
                    nc.vector._custom_dve(
                        TTR_OP, out=dd, in0=xt[:, k, :], in1=a_bc,
                        s0=0.0, s1=1.0, accum_out=ptd[:, j, c:c + 1])
                prods[i] = (p2d, p2p)

            def ws_reduce(i):
                b, c = records[i]
                p2d, p2p = prods.pop(i)
                pta = get_partials(b, "a", len(K_DVE_TT) + len(K_POOL_TT))
                da = dpool.tile([128, SC], F8, tag="da", name=f"da_{b}_{c}")
                for j in range(len(K_DVE_TT)):
                    nc.scalar.activation(da, p2d[:, j, :], AF.Copy,
                                         accum_out=pta[:, j, c:c + 1])
                for j in range(len(K_POOL_TT)):
                    nc.scalar.activation(da, p2p[:, j, :], AF.Copy,
                                         accum_out=pta[:, len(K_DVE_TT) + j,
                                                      c:c + 1])

            prods = {}

            def fin(b):
                acc = wpool.tile([128, KD], F32, tag="acc", name=f"acc_{b}")
                groups = [("d", (0, 1, 2)), ("a", (3, 4, 5, 6, 7))]  # noqa: keep
                for eng, ks in groups:
                    pt = partials.pop((b, eng))
                    nc.vector.tensor_reduce(
                        acc[:, ks[0]:ks[-1] + 1], pt, axis=AX.X, op=ALU.add)
                nc.vector.tensor_reduce(
                    zvals_sb[0:1, b:b + 1],
                    zcols_sb[0:1, b * NCH:(b + 1) * NCH], axis=AX.X, op=ALU.add)
                # transposing DMA writes acc[p, k] -> out[b, k*128+p] directly,
                # keeping the batch close off the PE/ACT queues entirely
                nc.sync.dma_start(
                    out_dram[b:b + 1, :].rearrange("1 (k p) -> p k", p=128), acc)

            # ---- software-pipelined issue ----
            emm(0)
            dv_ps = ps_m.tile([f, bp], F32, tag="dv", name="dv_ps")
            for k in range(KD):
                nc.tensor.matmul(dv_ps, wdt_sb[:, k, :], dht_sb[:, k, :],
                                 start=(k == 0), stop=(k == KD - 1))
            nc.vector.tensor_scalar_add(dvec_sb, dv_ps, web_sb)
            tanh_s(0)

            for i in range(1, NREC):
                if i + 2 < NREC:
                    xts[i + 2] = load_x(i + 2)
                emm(i)
                tanh_s(i)
                scores(i - 1)
                expa(i - 1)
                if i >= 2:
                    ws_produce(i - 2)
                if i >= 3:
                    ws_reduce(i - 3)
                j = i - 6
                if j >= 0 and j % NCH == NCH - 1:
                    fin(j // NCH)
            scores(NREC - 1)
            expa(NREC - 1)
            ws_produce(NREC - 2)
            ws_reduce(NREC - 3)
            ws_produce(NREC - 1)
            ws_reduce(NREC - 2)
            ws_reduce(NREC - 1)
            for b in range(bp):
                if (b, "d") in partials:
                    fin(b)
            nc.sync.dma_start(z_dram, zvals_sb)

    nc.finalize()
    return nc


_NC_CACHE = {}


def _get_nc(key, **kw):
    if key not in _NC_CACHE:
        _NC_CACHE[key] = build_nc(**kw)
    return _NC_CACHE[key]


def make_in_maps(e_hiddens, d_hiddens, We_w, We_b, Wd_w, v_w, v_b, n_cores=N_CORES):
    bp = e_hiddens.shape[0] // n_cores
    bf16 = ml_dtypes.bfloat16

    def arrange(m):  # [D, x] -> [128, KD*x], partition-major tiles
        dd, xx = m.shape
        return np.ascontiguousarray(
            m.reshape(dd // 128, 128, xx).transpose(1, 0, 2).reshape(128, -1))

    wet = arrange(np.ascontiguousarray(We_w.T)).astype(bf16)    # [128, KD*F]
    vrep = np.ascontiguousarray(
        np.repeat(v_w[0][:, None], 128, axis=1)).astype(bf16)   # [F, 128]
    vbb = np.full((128, 1), np.float32(v_b[0]), np.float32)
    web = np.ascontiguousarray(We_b[:, None]).astype(np.float32)
    wdt = arrange(np.ascontiguousarray(Wd_w.T))                 # [128, KD*F]
    maps = []
    for i in range(n_cores):
        xc = e_hiddens[i * bp:(i + 1) * bp]                     # [bp, S, D]
        # xt[b, c, p, k*SC + s'] = x[b, c*SC + s', k*128 + p]
        xt = np.ascontiguousarray(
            xc.reshape(bp, NCH, SC, KD, 128).transpose(0, 1, 4, 3, 2)
        ).astype(bf16).reshape(bp, NCH, 128, KD * SC)
        maps.append({
            "xt": xt,
            "wet": wet,
            "vrep": vrep,
            "vbb": vbb,
            "web": web,
            "wdt": wdt,
            "dht": arrange(np.ascontiguousarray(d_hiddens[i * bp:(i + 1) * bp].T)),
        })
    return maps


def kernel(e_hiddens, d_hiddens, length_mask, We_w, We_b, Wd_w, v_w, v_b,
           _trace=False):
    """Full inputs in, full output out.  length_mask is all-ones (the
    reference adds (1-mask)*1e-32, numerically a no-op)."""
    e_hiddens = np.asarray(e_hiddens, np.float32)
    d_hiddens = np.asarray(d_hiddens, np.float32)
    We_w = np.asarray(We_w, np.float32)
    We_b = np.asarray(We_b, np.float32)
    Wd_w = np.asarray(Wd_w, np.float32)
    v_w = np.asarray(v_w, np.float32)
    v_b = np.asarray(v_b, np.float32)

    nc = _get_nc("full")
    in_maps = make_in_maps(e_hiddens, d_hiddens, We_w, We_b, Wd_w, v_w, v_b)
    res = run_bass_kernel_spmd(nc, in_maps, list(range(N_CORES)), trace=_trace)
    outs = []
    for m in res.results:
        outs.append(m["out"].astype(np.float32) / m["z"].reshape(-1, 1))
    out = np.concatenate(outs, axis=0)
    if _trace:
        kernel.last_results = res
    return out


# revision 23
# speedup vs baseline: 1.0655x; 1.0655x over previous
"""Trainium2 Bass kernel for nn_AttnLayer (additive-attention pooling layer).

Reference computation (per batch b):
    e = e_hiddens @ We_w.T + We_b            # [S, F]
    d = Wd_w @ d_hiddens[b]                  # [F]
    h = tanh(d + e)                          # [S, F]
    s = h @ v_w[0] + v_b                     # [S]
    a = softmax(s)                           # [S]
    out[b] = a @ e_hiddens[b]                # [D]

Strategy (8 cores, data-parallel over batch B=32 -> 4 per core):
  x is pre-transposed ON HOST to [d-partition, s-free] bf16 tiles, so the
  device needs no PE transposes and no PSUM->SBUF copy stream, and DMA
  traffic halves (32 MiB/core).  Per 1024-long s-chunk:
    PE : e^T[f,s] = sum_k wet[dk,f]^T @ xt[dk,s]   (8 bf16 matmuls x 2 halves)
         scores replicated to all 128 partitions via a v-replicated
         stationary: sc[p,s] = sum_f vrep[f,p] h[f,s]  (2 matmuls)
    ACT: h = tanh(e^T + dvec_b)  ;  a = exp(sc + v_b) -> bf16 (accum_out
         gives the chunk's Z partial for free)
    WS : out_k += sum_s xt[dk,s] * a[s]  -- fused multiply+reduce
         (scalar_tensor_tensor accum_out) split DVE(k=0..2) / Pool(k=3..5),
         and tensor_tensor mult + ACT Copy-accum reduce for k=6..7,
         balancing all three engines under the DMA window.
  Softmax normalization (divide by Z) happens on host; the device returns
  unnormalized weighted sums plus per-batch Z.
"""

import numpy as np
import ml_dtypes

import concourse.bass as bass
import concourse.bacc as bacc
import concourse.mybir as mybir
import concourse.tile as tile
from concourse.bass_utils import run_bass_kernel_spmd
from concourse.dve_ops import TENSOR_TENSOR_REDUCE as TTR_OP

F32 = mybir.dt.float32
F32R = mybir.dt.float32r
BF16 = mybir.dt.bfloat16
F8 = mybir.dt.float8e4
AF = mybir.ActivationFunctionType
ALU = mybir.AluOpType
AX = mybir.AxisListType

N_CORES = 8
B, S, D, F = 32, 4096, 1024, 128
BP = B // N_CORES          # batches per core
KD = D // 128              # d-slices (partition groups)
SC = 1024                  # s-chunk (record granularity)
NCH = S // SC              # chunks per batch
NREC = BP * NCH            # records per core

# weighted-sum k-slice assignment, balanced to measured HW rates:
# DVE fused mult+reduce (TENSOR_TENSOR_REDUCE custom op) ~1.13-1.77us/slice,
# DVE plain mult ~0.6-1.2, Pool mult ~2.7-2.9, ACT Copy+accum reduce ~1.2,
# DVE tensor_reduce ~1.1 (writes no discard stream).
K_TTR = (0, 1, 2)           # DVE custom TENSOR_TENSOR_REDUCE (fused)
K_DVE_TT = (3, 4, 5)        # DVE tensor_tensor mult -> ACT Copy accum reduce
K_POOL_TT = (6, 7)          # Pool tensor_tensor mult -> ACT Copy accum reduce


def build_nc(bp=BP, s=S, d=D, f=F):
    nc = bacc.Bacc("TRN2", target_bir_lowering=False, debug=False)

    xt_dram = nc.dram_tensor("xt", [bp, NCH, 128, KD * SC], BF16,
                             kind="ExternalInput").ap()
    wet_dram = nc.dram_tensor("wet", [128, KD * f], BF16, kind="ExternalInput").ap()
    vrep_dram = nc.dram_tensor("vrep", [f, 128], BF16, kind="ExternalInput").ap()
    vbb_dram = nc.dram_tensor("vbb", [128, 1], F32, kind="ExternalInput").ap()
    web_dram = nc.dram_tensor("web", [f, 1], F32, kind="ExternalInput").ap()
    wdt_dram = nc.dram_tensor("wdt", [128, KD * f], F32R, kind="ExternalInput").ap()
    dht_dram = nc.dram_tensor("dht", [128, KD * bp], F32R, kind="ExternalInput").ap()
    out_dram = nc.dram_tensor("out", [bp, 128, KD], F32, kind="ExternalOutput").ap()
    z_dram = nc.dram_tensor("z", [1, bp], F32, kind="ExternalOutput").ap()

    with tile.TileContext(nc) as tc:
        with (
            tc.tile_pool(name="const", bufs=1) as const,
            tc.tile_pool(name="xpool", bufs=6) as xpool,
            tc.tile_pool(name="hpool", bufs=2) as hpool,
            tc.tile_pool(name="apool", bufs=3) as apool,
            tc.tile_pool(name="p2pool", bufs=3) as p2pool,
            tc.tile_pool(name="dpool", bufs=2) as dpool,
            tc.tile_pool(name="wpool", bufs=2) as wpool,
            tc.tile_pool(name="ps_e", bufs=2, space="PSUM") as ps_e,
            tc.tile_pool(name="ps_sc", bufs=1, space="PSUM") as ps_sc,
            tc.tile_pool(name="ps_m", bufs=1, space="PSUM") as ps_m,
        ):
            records = [(b, c) for b in range(bp) for c in range(NCH)]

            def load_x(i):
                b, c = records[i]
                xt = xpool.tile([128, KD, SC], BF16, tag="x", name=f"x_{b}_{c}")
                nc.sync.dma_start(
                    xt, xt_dram[b, c].rearrange("p (k s) -> p k s", k=KD))
                return xt

            xts = {i: load_x(i) for i in range(min(3, NREC))}

            # ---- constants (ordered by first use) ----
            wet_sb = const.tile([128, KD, f], BF16)
            nc.sync.dma_start(wet_sb, wet_dram.rearrange("p (k f) -> p k f", k=KD))
            vrep_sb = const.tile([f, 128], BF16)
            nc.sync.dma_start(vrep_sb, vrep_dram)
            vbb_sb = const.tile([128, 1], F32)
            nc.sync.dma_start(vbb_sb, vbb_dram)
            web_sb = const.tile([f, 1], F32)
            nc.sync.dma_start(web_sb, web_dram)
            wdt_sb = const.tile([128, KD, f], F32R)
            nc.sync.dma_start(wdt_sb, wdt_dram.rearrange("p (k f) -> p k f", k=KD))
            dht_sb = const.tile([128, KD, bp], F32R)
            nc.sync.dma_start(dht_sb, dht_dram.rearrange("p (k b) -> p k b", k=KD))
            dvec_sb = const.tile([f, bp], F32)
            zcols_sb = const.tile([128, bp * NCH], F32)
            zvals_sb = const.tile([1, bp], F32)

            state = {}

            def emm(i):
                b, c = records[i]
                e_ps = ps_e.tile([f, SC], F32, tag="e", name=f"e_{b}_{c}")
                xt = xts.pop(i)
                for k in range(KD):
                    for h2 in range(2):
                        sl = slice(h2 * 512, (h2 + 1) * 512)
                        nc.tensor.matmul(
                            e_ps[:, sl], wet_sb[:, k, :], xt[:, k, sl],
                            start=(k == 0), stop=(k == KD - 1),
                        )
                state[i] = {"e_ps": e_ps, "xt": xt}

            def tanh_s(i):
                b, c = records[i]
                e_ps = state[i].pop("e_ps")
                h_sb = hpool.tile([f, SC], BF16, tag="h", name=f"h_{b}_{c}")
                nc.scalar.activation(h_sb, e_ps, AF.Tanh, bias=dvec_sb[:, b:b + 1])
                state[i]["h"] = h_sb

            def scores(i):
                b, c = records[i]
                h_sb = state[i].pop("h")
                sc_ps = ps_sc.tile([128, SC], F32, tag="sc", name=f"sc_{b}_{c}")
                for h2 in range(2):
                    sl = slice(h2 * 512, (h2 + 1) * 512)
                    nc.tensor.matmul(sc_ps[:, sl], vrep_sb, h_sb[:, sl],
                                     start=True, stop=True)
                state[i]["sc_ps"] = sc_ps

            def expa(i):
                b, c = records[i]
                sc_ps = state[i].pop("sc_ps")
                a_bc = apool.tile([128, SC], BF16, tag="a", name=f"a_{b}_{c}")
                nc.scalar.activation(a_bc, sc_ps, AF.Exp, bias=vbb_sb,
                                     accum_out=zcols_sb[:, b * NCH + c:b * NCH + c + 1])
                state[i]["a"] = a_bc

            def get_partials(b, eng, n):
                key = (b, eng)
                if key not in partials:
                    partials[key] = wpool.tile(
                        [128, n, NCH], F32, tag=f"pt_{eng}", name=f"pt_{eng}_{b}")
                return partials[key]

            partials = {}

            def ws_produce(i):
                b, c = records[i]
                st = state.pop(i)
                xt, a_bc = st["xt"], st["a"]
                ptd = get_partials(b, "d", len(K_TTR))
                # Pool first (slowest producer), then DVE mults (feed ACT
                # early), then the fused DVE reduces.
                p2p = p2pool.tile([128, len(K_POOL_TT), SC], BF16, tag="p2p",
                                  name=f"p2p_{b}_{c}")
                for j, k in enumerate(K_POOL_TT):
                    nc.gpsimd.tensor_tensor(p2p[:, j, :], xt[:, k, :], a_bc,
                                            op=ALU.mult)
                p2d = p2pool.tile([128, len(K_DVE_TT), SC], BF16, tag="p2d",
                                  name=f"p2d_{b}_{c}")
                for j, k in enumerate(K_DVE_TT):
                    nc.vector.tensor_tensor(p2d[:, j, :], xt[:, k, :], a_bc,
                                            op=ALU.mult)
                dd = dpool.tile([128, SC], F8, tag="dd", name=f"dd_{b}_{c}")
                for j, k in enumerate(K_TTR):
                    nc.vector._custom_dve(
                        TTR_OP, out=dd, in0=xt[:, k, :], in1=a_bc,
                        s0=0.0, s1=1.0, accum_out=ptd[:, j, c:c + 1])
                prods[i] = (p2d, p2p)

            def ws_reduce(i):
                b, c = records[i]
                p2d, p2p = prods.pop(i)
                pta = get_partials(b, "a", len(K_DVE_TT) + len(K_POOL_TT))
                da = dpool.tile([128, SC], F8, tag="da", name=f"da_{b}_{c}")
                for j in range(len(K_DVE_TT)):
                    nc.scalar.activation(da, p2d[:, j, :], AF.Copy,
                                         accum_out=pta[:, j, c:c + 1])
                for j in range(len(K_POOL_TT)):
                    nc.scalar.activation(da, p2p[:, j, :], AF.Copy,
                                         accum_out=pta[:, len(K_DVE_TT) + j,
                                                      c:c + 1])

            prods = {}

            def fin(b):
                acc = wpool.tile([128, KD], F32, tag="acc", name=f"acc_{b}")
                groups = [("d", (0, 1, 2)), ("a", (3, 4, 5, 6, 7))]  # noqa: keep
                for eng, ks in groups:
                    pt = partials.pop((b, eng))
                    nc.vector.tensor_reduce(
                        acc[:, ks[0]:ks[-1] + 1], pt, axis=AX.X, op=ALU.add)
                nc.vector.tensor_reduce(
                    zvals_sb[0:1, b:b + 1],
                    zcols_sb[0:1, b * NCH:(b + 1) * NCH], axis=AX.X, op=ALU.add)
                # contiguous DMA in [p, k] layout; host re-orders to [d]
                nc.sync.dma_start(out_dram[b], acc)

            # ---- software-pipelined issue ----
            emm(0)
            dv_ps = ps_m.tile([f, bp], F32, tag="dv", name="dv_ps")
            for k in range(KD):
                nc.tensor.matmul(dv_ps, wdt_sb[:, k, :], dht_sb[:, k, :],
                                 start=(k == 0), stop=(k == KD - 1))
            nc.vector.tensor_scalar_add(dvec_sb, dv_ps, web_sb)
            tanh_s(0)

            for i in range(1, NREC):
                if i + 2 < NREC:
                    xts[i + 2] = load_x(i + 2)
                emm(i)
                tanh_s(i)
                scores(i - 1)
                expa(i - 1)
                if i >= 2:
                    ws_produce(i - 2)
                if i >= 3:
                    ws_reduce(i - 3)
                j = i - 6
                if j >= 0 and j % NCH == NCH - 1:
                    fin(j // NCH)
            scores(NREC - 1)
            expa(NREC - 1)
            ws_produce(NREC - 2)
            ws_reduce(NREC - 3)
            ws_produce(NREC - 1)
            ws_reduce(NREC - 2)
            ws_reduce(NREC - 1)
            for b in range(bp):
                if (b, "d") in partials:
                    fin(b)
            nc.sync.dma_start(z_dram, zvals_sb)

    nc.finalize()
    return nc


_NC_CACHE = {}


def _get_nc(key, **kw):
    if key not in _NC_CACHE:
        _NC_CACHE[key] = build_nc(**kw)
    return _NC_CACHE[key]


def make_in_maps(e_hiddens, d_hiddens, We_w, We_b, Wd_w, v_w, v_b, n_cores=N_CORES):
    bp = e_hiddens.shape[0] // n_cores
    bf16 = ml_dtypes.bfloat16

    def arrange(m):  # [D, x] -> [128, KD*x], partition-major tiles
        dd, xx = m.shape
        return np.ascontiguousarray(
            m.reshape(dd // 128, 128, xx).transpose(1, 0, 2).reshape(128, -1))

    wet = arrange(np.ascontiguousarray(We_w.T)).astype(bf16)    # [128, KD*F]
    vrep = np.ascontiguousarray(
        np.repeat(v_w[0][:, None], 128, axis=1)).astype(bf16)   # [F, 128]
    vbb = np.full((128, 1), np.float32(v_b[0]), np.float32)
    web = np.ascontiguousarray(We_b[:, None]).astype(np.float32)
    wdt = arrange(np.ascontiguousarray(Wd_w.T))                 # [128, KD*F]
    maps = []
    for i in range(n_cores):
        xc = e_hiddens[i * bp:(i + 1) * bp]                     # [bp, S, D]
        # xt[b, c, p, k*SC + s'] = x[b, c*SC + s', k*128 + p]
        xt = np.ascontiguousarray(
            xc.reshape(bp, NCH, SC, KD, 128).transpose(0, 1, 4, 3, 2)
        ).astype(bf16).reshape(bp, NCH, 128, KD * SC)
        maps.append({
            "xt": xt,
            "wet": wet,
            "vrep": vrep,
            "vbb": vbb,
            "web": web,
            "wdt": wdt,
            "dht": arrange(np.ascontiguousarray(d_hiddens[i * bp:(i + 1) * bp].T)),
        })
    return maps


def kernel(e_hiddens, d_hiddens, length_mask, We_w, We_b, Wd_w, v_w, v_b,
           _trace=False):
    """Full inputs in, full output out.  length_mask is all-ones (the
    reference adds (1-mask)*1e-32, numerically a no-op)."""
    e_hiddens = np.asarray(e_hiddens, np.float32)
    d_hiddens = np.asarray(d_hiddens, np.float32)
    We_w = np.asarray(We_w, np.float32)
    We_b = np.asarray(We_b, np.float32)
    Wd_w = np.asarray(Wd_w, np.float32)
    v_w = np.asarray(v_w, np.float32)
    v_b = np.asarray(v_b, np.float32)

    nc = _get_nc("full")
    in_maps = make_in_maps(e_hiddens, d_hiddens, We_w, We_b, Wd_w, v_w, v_b)
    res = run_bass_kernel_spmd(nc, in_maps, list(range(N_CORES)), trace=_trace)
    outs = []
    for m in res.results:
        o = m["out"].transpose(0, 2, 1).reshape(BP, D)  # [bp,p,k] -> [bp,d]
        outs.append(o.astype(np.float32) / m["z"].reshape(-1, 1))
    out = np.concatenate(outs, axis=0)
    if _trace:
        kernel.last_results = res
    return out
